# revision 13
# baseline (speedup 1.0000x reference)
"""Trainium2 Bass kernel for nn_MergeNN (retrieval_knn).

Math (reference):
  match_idx = argmin_n ||x_i - F_star_n||^2                       [K]
  per branch b: xt = feats_b[match_idx]; y = xt@W_b + b_b
                cls = argmin_c ||y - uls_c||^2
                w   = exp(-||xt_i - feats_b_j||^2) * [lab_b_j == cls_i]
                out_b = (w @ Y_star) / w.sum(1)
  out = (out_1 + out_2) / 2

Implementation: N=60000 sharded over 8 cores (7500 rows/core, padded to
7552 = 59*128).  Two SPMD launches; the big [K,784]x[784,N] products run
on the PE in fp8(e4m3) DoubleRow mode (contraction 256/matmul, 0.5
cycles/row; quantization error on the O(1) squared distances is ~6e-4,
far under the ~0.23 argmin margin and ~0.1% on the RBF weights).

L1 (argmin): per core s[i,j] = x_i . f_j - ||f_j||^2/2; fp8 DR matmuls
  for the data rows + one bf16 K=1 matmul adding the -||f||^2/2 row;
  per-query top-1 value+index over the local shard (DVE max/max_index).
  Host combines the 8 (val, idx) candidates -> global match_idx, gathers
  xt rows, computes tiny y/cls on host (fp32, exact argmin semantics).

L2 (weights+aggregate): per core/branch t[n,q] = exp(2 * xt_q . f_n)
  (one ACT op from PSUM, bf16 out); P[q, 11*c+m] += t[n,q]*T[n, 11*c+m]
  with T[n, 11*c+m] = exp(-||f_n||^2) * [lab_n == c] * [Y_n | 1]_m  --
  the f-norm factor, label mask, Y aggregation and weight row-sum are
  all folded into one bf16 matmul.  The per-query factor exp(-||xt||^2)
  cancels in the final num/den division, so it is dropped entirely.
  Host sums the per-core partials, selects the 11-column block by cls,
  divides, and averages the branches.
"""

import numpy as np
import ml_dtypes

import concourse.bass as bass
import concourse.mybir as mybir
import concourse.tile as tile
from concourse import bacc
from concourse.bass_utils import run_bass_kernel_spmd

BF16 = ml_dtypes.bfloat16
FP8 = ml_dtypes.float8_e4m3
F32 = np.float32

NCORES = 8
N, K, D, C = 60000, 1024, 784, 10
NSH = N // NCORES            # 7500 rows per core
NT = 59                      # n tiles of 128
NPAD = NT * 128              # 7552
DP = 1024                    # contraction rows padded for DoubleRow (8*128)
DJ = 8                       # fp8 k-subtiles
QT = K // 128                # 8 query tiles
CC = (C + 1) * C             # 110 = 10 classes x (10 label cols + 1 sum col)
NCH = (NPAD + 511) // 512    # 15 free-dim chunks in L1
NEG = -1.0e30
DR = mybir.MatmulPerfMode.DoubleRow

_cache = {}


def _dr_pack(a):
    """[D, M] fp32 -> DoubleRow-packed fp8 [128, DJ*M] (contraction padded
    to DP rows; layout [p, j, m] = row j*128+p)."""
    d, m = a.shape
    pad = np.zeros((DP, m), F32)
    pad[:d] = a
    return np.ascontiguousarray(
        pad.reshape(DJ, 128, m).transpose(1, 0, 2)).astype(FP8).reshape(128, DJ * m)


def _build_l1():
    nc = bacc.Bacc("TRN2", debug=False)
    xdr = nc.dram_tensor("xdr", [128, DJ * K], mybir.dt.float8e4,
                         kind="ExternalInput").ap().rearrange("p (j m) -> p j m", j=DJ)
    fdr = nc.dram_tensor("fdr", [128, DJ * NPAD], mybir.dt.float8e4,
                         kind="ExternalInput").ap().rearrange("p (j m) -> p j m", j=DJ)
    fnrow = nc.dram_tensor("fnrow", [1, NPAD], mybir.dt.bfloat16,
                           kind="ExternalInput").ap()
    maxv = nc.dram_tensor("maxv", [K], mybir.dt.bfloat16, kind="ExternalOutput").ap()
    amax = nc.dram_tensor("amax", [K], mybir.dt.uint32, kind="ExternalOutput").ap()

    with tile.TileContext(nc) as tc:
        with (
            tc.sbuf_pool(name="tab", bufs=1) as tab,
            tc.sbuf_pool(name="work", bufs=2) as work,
            tc.sbuf_pool(name="outp", bufs=2) as outp,
            tc.psum_pool(name="ps", bufs=4) as ps,
        ):
            x_sb = tab.tile([128, DJ, K], mybir.dt.float8e4, name="x_sb")
            nc.sync.dma_start(x_sb[:], xdr)
            fn_sb = tab.tile([1, NPAD], mybir.dt.bfloat16, name="fn_sb")
            nc.sync.dma_start(fn_sb[:], fnrow)
            ones = tab.tile([1, K], mybir.dt.bfloat16, name="ones")
            nc.gpsimd.memset(ones[:], 1.0)
            # split the big table load so matmuls start after the first region
            REG = 2048 // 512
            f_sb = []
            for r in range(4):
                lo, hi = r * 2048, min((r + 1) * 2048, NPAD)
                ft = tab.tile([128, DJ, hi - lo], mybir.dt.float8e4,
                              tag=f"f{r}", name=f"f{r}")
                nc.sync.dma_start(ft[:], fdr[:, :, lo:hi])
                f_sb.append(ft)

            for q in range(QT):
                d_sb = work.tile([128, NPAD], mybir.dt.bfloat16, tag="d", name="d")
                for ch in range(NCH):
                    w = min(512, NPAD - ch * 512)
                    r, lch = ch // REG, ch % REG
                    pt = ps.tile([128, 512], mybir.dt.float32, tag="pt", name="pt")
                    for j in range(DJ // 2):
                        nc.tensor.matmul(
                            pt[:, :w],
                            x_sb[:, 2 * j:2 * j + 2, q * 128:(q + 1) * 128],
                            f_sb[r][:, 2 * j:2 * j + 2, lch * 512:lch * 512 + w],
                            start=(j == 0), stop=False, perf_mode=DR)
                    nc.tensor.matmul(
                        pt[:, :w],
                        ones[:, q * 128:(q + 1) * 128],
                        fn_sb[:, ch * 512:ch * 512 + w],
                        start=False, stop=True)
                    nc.scalar.copy(d_sb[:, ch * 512:ch * 512 + w], pt[:, :w])
                mx = outp.tile([128, 8], mybir.dt.bfloat16, tag="mx", name="mx")
                ix = outp.tile([128, 8], mybir.dt.uint32, tag="ix", name="ix")
                nc.vector.max(mx[:], d_sb[:])
                nc.vector.max_index(ix[:], mx[:], d_sb[:])
                nc.sync.dma_start(maxv[q * 128:(q + 1) * 128], mx[:, 0:1])
                nc.sync.dma_start(amax[q * 128:(q + 1) * 128], ix[:, 0:1])
    nc.compile()
    return nc


def _build_l2():
    nc = bacc.Bacc("TRN2", debug=False)
    ins = {}
    outs = {}
    for b in (1, 2):
        ins[f"xtdr{b}"] = nc.dram_tensor(
            f"xtdr{b}", [128, DJ * K], mybir.dt.float8e4,
            kind="ExternalInput").ap().rearrange("p (j m) -> p j m", j=DJ)
        ins[f"fdr{b}"] = nc.dram_tensor(
            f"fdr{b}", [128, DJ * NPAD], mybir.dt.float8e4,
            kind="ExternalInput").ap().rearrange("p (j m) -> p j m", j=DJ)
        ins[f"Tt{b}"] = nc.dram_tensor(
            f"Tt{b}", [128, NT * CC], mybir.dt.bfloat16, kind="ExternalInput").ap()
        outs[b] = nc.dram_tensor(
            f"P{b}", [K, CC], mybir.dt.float32, kind="ExternalOutput").ap()

    NTR = [15, 15, 15, 14]  # n-tile split per DMA region
    with tile.TileContext(nc) as tc:
        with (
            tc.sbuf_pool(name="tab", bufs=2) as tab,
            tc.sbuf_pool(name="work", bufs=3) as work,
            tc.sbuf_pool(name="outp", bufs=4) as outp,
            tc.psum_pool(name="ps_t", bufs=3) as ps_t,
            tc.psum_pool(name="ps_p", bufs=2) as ps_p,
        ):
            for b in (1, 2):
                xt_sb = tab.tile([128, DJ, K], mybir.dt.float8e4,
                                 tag="x", name=f"x{b}")
                nc.sync.dma_start(xt_sb[:], ins[f"xtdr{b}"])
                T_sb = tab.tile([128, NT, CC], mybir.dt.bfloat16, tag="T", name=f"T{b}")
                nc.sync.dma_start(
                    T_sb[:], ins[f"Tt{b}"].rearrange("p (n c) -> p n c", c=CC))
                f_sb = []
                for r in range(4):
                    lo = sum(NTR[:r]) * 128
                    hi = lo + NTR[r] * 128
                    ft = tab.tile([128, DJ, hi - lo], mybir.dt.float8e4,
                                  tag=f"f{r}", name=f"f{b}_{r}")
                    nc.sync.dma_start(ft[:], ins[f"fdr{b}"][:, :, lo:hi])
                    f_sb.append(ft)

                for qh in range(2):
                    # all 4 query-subtile accumulators packed in one PSUM bank
                    p_ps = ps_p.tile([128, 4, CC], mybir.dt.float32,
                                     tag="P", name=f"P{b}_{qh}")
                    for nt in range(NT):
                        r = min(nt // 15, 3)
                        lnt = nt - sum(NTR[:r])
                        pt = ps_t.tile([128, 512], mybir.dt.float32, tag="t", name="t")
                        for j in range(DJ // 2):
                            nc.tensor.matmul(
                                pt[:],
                                f_sb[r][:, 2 * j:2 * j + 2, lnt * 128:(lnt + 1) * 128],
                                xt_sb[:, 2 * j:2 * j + 2, qh * 512:(qh + 1) * 512],
                                start=(j == 0), stop=(j == DJ // 2 - 1), perf_mode=DR)
                        t_sb = work.tile([128, 512], mybir.dt.bfloat16,
                                         tag="t_sb", name="t_sb")
                        nc.scalar.activation(
                            t_sb[:], pt[:],
                            mybir.ActivationFunctionType.Exp, scale=2.0)
                        for qs in range(4):
                            # one accumulation group per PSUM bank: start
                            # zeroes the whole 2KB zero-region, stop ends it
                            nc.tensor.matmul(
                                p_ps[:, qs, :],
                                t_sb[:, qs * 128:(qs + 1) * 128],
                                T_sb[:, nt, :],
                                start=(nt == 0 and qs == 0),
                                stop=(nt == NT - 1 and qs == 3),
                            )
                    o = outp.tile([128, 4, CC], mybir.dt.float32, tag="o", name="o")
                    nc.scalar.copy(o[:], p_ps[:])
                    nc.sync.dma_start(
                        outs[b][qh * 512:(qh + 1) * 512, :].rearrange(
                            "(a p) c -> p a c", p=128),
                        o[:])
    nc.compile()
    return nc


def _get(name, builder):
    if name not in _cache:
        _cache[name] = builder()
    return _cache[name]


def _run_spmd(nc, in_maps, core_ids):
    """run_bass_kernel_spmd with retry: the device occasionally throws a
    transient NRT_EXEC_UNIT_UNRECOVERABLE.  Once that happens the PJRT
    client is poisoned, so tear down the jax backend (a fresh client to
    the axon terminal recovers) before retrying."""
    last = None
    for attempt in range(4):
        try:
            return run_bass_kernel_spmd(nc, in_maps, core_ids)
        except Exception as e:  # noqa: BLE001
            last = e
            import time
            time.sleep(3.0 * (attempt + 1))
            try:
                import jax
                from jax._src import xla_bridge as xb
                jax.clear_caches()
                xb._clear_backends()
            except Exception:
                pass
    raise last


def _sqdist_np(a, b):
    return ((a * a).sum(-1)[:, None] + (b * b).sum(-1)[None, :]
            - 2.0 * (a @ b.T)).astype(F32)


def kernel(**inputs):
    x = np.ascontiguousarray(np.asarray(inputs["x"], F32))
    F_star = np.asarray(inputs["F_star"], F32)
    Y_star = np.asarray(inputs["Y_star"], F32)
    feats = [np.asarray(inputs["feats1"], F32), np.asarray(inputs["feats2"], F32)]
    uls = [np.asarray(inputs["uls1"], F32), np.asarray(inputs["uls2"], F32)]
    Ws = [np.asarray(inputs["W1"], F32), np.asarray(inputs["W2"], F32)]
    bs = [np.asarray(inputs["b1"], F32), np.asarray(inputs["b2"], F32)]
    labs = [np.asarray(inputs["lab1"]).astype(np.int64),
            np.asarray(inputs["lab2"]).astype(np.int64)]

    core_ids = list(range(NCORES))
    from concurrent.futures import ThreadPoolExecutor
    if "pool" not in _cache:
        _cache["pool"] = ThreadPoolExecutor(16)
    pool = _cache["pool"]

    # ---------------- L1: global argmin over N ----------------
    nc1 = _get("l1", _build_l1)

    xdr = _dr_pack(x.T)
    fn = np.einsum("nd,nd->n", F_star, F_star, dtype=np.float32)

    def prep1(c):
        Fc = np.zeros((D, NPAD), F32)
        Fc[:, :NSH] = F_star[c * NSH:(c + 1) * NSH].T
        fnrow = np.full((1, NPAD), NEG, F32)
        fnrow[0, :NSH] = -0.5 * fn[c * NSH:(c + 1) * NSH]
        return {"xdr": xdr, "fdr": _dr_pack(Fc), "fnrow": fnrow.astype(BF16)}

    fut1 = [pool.submit(prep1, c) for c in range(NCORES)]

    # L2 table prep is independent of the L1 result -> overlap with L1 run
    def prep2(bi):
        fb = feats[bi]
        fnb = np.einsum("nd,nd->n", fb, fb, dtype=np.float32)
        # aggregation table: T[j, 11*c+m] = e^{-|f_j|^2} [lab_j == c] [Y_j|1]_m
        Yext = np.concatenate([Y_star, np.ones((N, 1), F32)], axis=1)  # [N, 11]
        Yext = Yext * np.exp(-fnb)[:, None]
        Tfull = np.zeros((N, CC), F32)
        cols = (labs[bi][:, None] * (C + 1) + np.arange(C + 1)[None, :])
        np.put_along_axis(Tfull, cols, Yext, axis=1)

        def core_tabs(c):
            Fc = np.zeros((D, NPAD), F32)
            Fc[:, :NSH] = fb[c * NSH:(c + 1) * NSH].T
            Tc = np.zeros((NPAD, CC), F32)
            Tc[:NSH] = Tfull[c * NSH:(c + 1) * NSH]
            Tt = np.ascontiguousarray(
                Tc.astype(BF16).reshape(NT, 128, CC).transpose(1, 0, 2)
            ).reshape(128, NT * CC)
            return _dr_pack(Fc), Tt
        return [core_tabs(c) for c in range(NCORES)]

    fut2 = [pool.submit(prep2, bi) for bi in range(2)]

    in_maps1 = [f.result() for f in fut1]
    res1 = _run_spmd(nc1, in_maps1, core_ids)
    allv = np.stack([res1.results[c]["maxv"].astype(F32) for c in range(NCORES)])
    alli = np.stack([res1.results[c]["amax"].astype(np.int64) for c in range(NCORES)])
    best_core = np.argmax(allv, axis=0)                       # first max wins ties
    match_idx = best_core * NSH + alli[best_core, np.arange(K)]

    # ---------------- host: tiny per-branch prep ----------------
    nc2 = _get("l2", _build_l2)
    in_maps2 = [dict() for _ in range(NCORES)]
    cls_b = []
    for bi in range(2):
        fb = feats[bi]
        xt = np.ascontiguousarray(fb[match_idx])              # [K, D] fp32
        y = xt @ Ws[bi] + bs[bi]
        cls = np.argmin(_sqdist_np(y, uls[bi]), axis=1)       # [K]
        cls_b.append(cls)
        xtdr = _dr_pack(xt.T)
        tabs = fut2[bi].result()
        for c in range(NCORES):
            in_maps2[c][f"xtdr{bi + 1}"] = xtdr
            in_maps2[c][f"fdr{bi + 1}"] = tabs[c][0]
            in_maps2[c][f"Tt{bi + 1}"] = tabs[c][1]

    # ---------------- L2: masked RBF aggregation ----------------
    res2 = _run_spmd(nc2, in_maps2, core_ids)

    out = np.zeros((K, C), F32)
    rows = np.arange(K)
    for bi in range(2):
        P = np.zeros((K, CC), F32)
        for c in range(NCORES):
            P += res2.results[c][f"P{bi + 1}"]
        base = cls_b[bi] * (C + 1)
        num = P[rows[:, None], base[:, None] + np.arange(C)[None, :]]
        den = P[rows, base + C]
        out += num / den[:, None]
    return (0.5 * out).astype(F32)
